# revision 15
# baseline (speedup 1.0000x reference)
"""Trainium2 Bass kernel for nn_EncoderSimilarity (block-cosine similarity).

sims[a,b] = sum over block-granularities {128, 256} of
            sum_t max_v ( l2norm(img_block_v) . l2norm(cap_block_t) )

Sharding: img rows (axis a) split 8 ways across cores, cap replicated;
each core computes its [256, 2048] slice of sims.

v3 device algorithm (per core).  Same math as v2 (max-of-8 restructured
into relu'd pair differences + PSUM-accumulated deltas; t-sums factored
through the PE via capsums and identity matmuls), re-engineered for
engine overlap:

  * PSUM is carved into 2-bank unit tiles ([128,2,512] f32) from a
    bufs=3 rotation (6 banks) + 1 acc bank + 1 transpose bank.  p3 is
    allocated FIRST in each tq so u2 reuses its slot right after the
    early ACT relu-drain; the PE streams tq(k+1) seeds while ACT/DVE
    drain tq(k).  v2 used all 8 banks per tq -> no cross-tq overlap.
  * Prep engine assignment keeps the DVE/ACT main path free of
    head-of-line blocking: q1-3 normalization (squares, reduces,
    scale-muls) runs entirely on GPSIMD, q0+img prep splits across
    ACT/DVE/GP in the prologue, capsums q0 on GP / q1-3 on DVE emitted
    one quarter ahead of use.
"""
import sys

if "/opt/trn_rl_repo" not in sys.path:
    sys.path.insert(0, "/opt/trn_rl_repo")

from contextlib import ExitStack

import numpy as np

N_CORES = 8
A, B, C = 2048, 2048, 1024
A_PER = A // N_CORES          # 256 img rows per core
NQ = 4                        # b processed in quarters of 512
BQ = B // NQ                  # 512


def _build_kernel():
    import concourse.bass as bass
    import concourse.tile as tile
    from concourse import mybir

    F32 = mybir.dt.float32
    BF16 = mybir.dt.bfloat16
    Alu = mybir.AluOpType
    Act = mybir.ActivationFunctionType
    Ax = mybir.AxisListType

    nc = bass.Bass(
        trn_type="TRN2",
        target_bir_lowering=False,
        debug=False,
        num_devices=N_CORES,
    )
    img_d = nc.dram_tensor("img", [A_PER, C], F32, kind="ExternalInput").ap()
    cap_d = nc.dram_tensor("cap", [B, C], F32, kind="ExternalInput").ap()
    ident_d = nc.dram_tensor("ident", [128, 128], BF16, kind="ExternalInput").ap()
    out_d = nc.dram_tensor("sims", [A_PER, B], F32, kind="ExternalOutput").ap()

    with tile.TileContext(nc) as tc, ExitStack() as ctx:
        _body(ctx, tc, out_d, img_d, cap_d, ident_d, F32, BF16, Alu, Act, Ax)
    return nc


def _body(ctx, tc, out_d, img_d, cap_d, ident_d, F32, BF16, Alu, Act, Ax):
    nc = tc.nc

    dram = ctx.enter_context(tc.tile_pool(name="dram", bufs=1, space="DRAM"))
    persist = ctx.enter_context(tc.tile_pool(name="persist", bufs=1))
    norm = ctx.enter_context(tc.tile_pool(name="norm", bufs=2))
    small = ctx.enter_context(tc.tile_pool(name="small", bufs=3))
    stage = ctx.enter_context(tc.tile_pool(name="stage", bufs=2))
    drain = ctx.enter_context(tc.tile_pool(name="drain", bufs=2))
    csum = ctx.enter_context(tc.tile_pool(name="csum", bufs=1))
    prep = ctx.enter_context(tc.tile_pool(name="prep", bufs=3))
    capin = ctx.enter_context(tc.tile_pool(name="capin", bufs=6))
    # PSUM: "u" 3 x 2 banks + acc 1 bank + transpose 1 bank = 8 banks
    psum = ctx.enter_context(tc.tile_pool(name="psum", bufs=3, space="PSUM"))
    psacc = ctx.enter_context(tc.tile_pool(name="psacc", bufs=2, space="PSUM"))

    ident = persist.tile([128, 128], BF16, tag="ident")
    nc.sync.dma_start(ident[:], ident_d[:])

    # ---------------- normalization helper (natural [n, c] layout) -------------
    def norm_sums(x_f32, nm, sq_act):
        """square + per-block sums -> sq tile.  Separate tile tags per path:
        sharing a ring between the prologue (ACT/DVE) and steady-state (GP)
        paths creates write-after-read hazards that stall GP behind the
        whole prologue DVE chain."""
        tag = "sq" if sq_act else "sqg"
        sq = norm.tile([128, C], F32, tag=tag, name=f"sq_{nm}", bufs=2)
        if sq_act:
            nc.scalar.activation(sq[:], x_f32[:], Act.Square)
        else:
            nc.gpsimd.tensor_mul(sq[:], x_f32[:], x_f32[:])
        return sq

    def norm_reduce(sq, nm):
        s12 = small.tile([128, 12], F32, tag="s12", name=f"s12_{nm}")
        nc.vector.reduce_sum(
            s12[:, 0:8], sq.rearrange("p (j c) -> p j c", c=128), axis=Ax.X
        )
        nc.vector.tensor_tensor(
            s12[:, 8:12],
            s12.rearrange("p (k two) -> p k two", two=2)[:, 0:4, 0],
            s12.rearrange("p (k two) -> p k two", two=2)[:, 0:4, 1],
            op=Alu.add,
        )
        return s12

    def norm_apply(x_f32, s12, n128_out, n256_out, nm, mul_gp):
        rt = small.tile([128, 12], F32, tag="rt", name=f"rt_{nm}")
        nc.scalar.activation(rt[:], s12[:], Act.Sqrt)
        inv = small.tile([128, 12], F32, tag="inv", name=f"inv_{nm}")
        nc.vector.reciprocal(inv[:], rt[:])
        eng = nc.gpsimd if mul_gp else nc.vector
        eng.tensor_mul(
            n128_out.rearrange("p (j c) -> p j c", c=128),
            x_f32.rearrange("p (j c) -> p j c", c=128),
            inv[:, 0:8].unsqueeze(2).to_broadcast((128, 8, 128)),
        )
        eng.tensor_mul(
            n256_out.rearrange("p (k c) -> p k c", c=256),
            x_f32.rearrange("p (k c) -> p k c", c=256),
            inv[:, 8:12].unsqueeze(2).to_broadcast((128, 4, 256)),
        )

    def normalize_tile(x_f32, n128_out, n256_out, nm, mode, pool=None):
        """Prologue path: all three phases back-to-back."""
        sq = norm_sums(x_f32, nm, sq_act=True)
        s12 = norm_reduce(sq, nm)
        norm_apply(x_f32, s12, n128_out, n256_out, nm, mul_gp=(mode == "gp"))

    # ---------------- img prep -> transposed bf16 weight tiles -----------------
    # w128T slots: 0..3 = D_p = n128[2p]-n128[2p+1]; 4..6 = wd_j = n128[2j+1]-
    # n128[7]; 7 = base n128[7].
    # w256T slots: 0..3 = D'_i half h (2i+h); 4..5 = wd' h; 6..7 = base' h.
    w128T = [persist.tile([128, 8, 128], BF16, tag=f"w128T_{at}", name=f"w128T_{at}") for at in range(2)]
    w256T = [persist.tile([128, 8, 128], BF16, tag=f"w256T_{at}", name=f"w256T_{at}") for at in range(2)]

    def img_prep(at):
        x = norm.tile([128, C], F32, tag="img_in", name=f"img_in_{at}")
        nc.sync.dma_start(x[:], img_d[at * 128:(at + 1) * 128, :])
        n128 = norm.tile([128, C], BF16, tag="img_n128", name=f"img_n128_{at}")
        n256 = norm.tile([128, C], BF16, tag="img_n256", name=f"img_n256_{at}")
        normalize_tile(x, n128, n256, f"img{at}", mode="fast")

        d128 = norm.tile([128, 8, 128], BF16, tag="d128", name=f"d128_{at}")
        v128 = n128.rearrange("p (v c) -> p v c", c=128)
        nc.vector.tensor_tensor(d128[:, 0:4, :], v128[:, 0::2, :], v128[:, 1::2, :],
                                op=Alu.subtract)
        nc.vector.tensor_tensor(
            d128[:, 4:7, :], v128[:, 1:7:2, :],
            v128[:, 7:8, :].to_broadcast((128, 3, 128)), op=Alu.subtract)
        nc.vector.tensor_copy(d128[:, 7, :], v128[:, 7, :])

        d256 = norm.tile([128, 8, 128], BF16, tag="d256", name=f"d256_{at}")
        v256 = n256.rearrange("p (v c) -> p v c", c=256)
        nc.vector.tensor_tensor(
            d256.rearrange("p (i h) c -> p i (h c)", h=2)[:, 0:2, :],
            v256[:, 0::2, :], v256[:, 1::2, :], op=Alu.subtract)
        nc.vector.tensor_tensor(d256[:, 4:6, :].rearrange("p h c -> p (h c)"),
                                v256[:, 1, :], v256[:, 3, :], op=Alu.subtract)
        nc.vector.tensor_copy(d256[:, 6:8, :].rearrange("p h c -> p (h c)"),
                              v256[:, 3, :])

        for gi, (src, dstT) in enumerate(((d128, w128T[at]), (d256, w256T[at]))):
            for jg in range(2):
                pt = psum.tile([128, 4, 128], BF16, tag="u",
                               name=f"ptw_{at}_{gi}_{jg}")
                for k in range(4):
                    nc.tensor.transpose(pt[:, k, :], src[:, jg * 4 + k, :], ident[:])
                if (gi + jg) % 2 == 0:
                    nc.vector.tensor_copy(dstT[:, jg * 4:(jg + 1) * 4, :], pt[:])
                else:
                    nc.scalar.copy(dstT[:, jg * 4:(jg + 1) * 4, :], pt[:])

    # ---------------- cap prep ------------------------------------------------
    scr_c128 = dram.tile([B, C], BF16, tag="scr_c128")
    scr_c256 = dram.tile([B, C], BF16, tag="scr_c256")

    capT128 = {}
    capT256 = {}
    for q in range(NQ):
        capT128[q] = persist.tile([128, 8, BQ], BF16, tag=f"capT128_{q % 2}",
                                  name=f"capT128_{q}")
        capT256[q] = persist.tile([128, 8, BQ], BF16, tag=f"capT256_{q % 2}",
                                  name=f"capT256_{q}")

    def cap_norm_tile(q, r, mode):
        """Normalize cap row-tile (q, r); q0 PE-transposes from SBUF,
        q>0 writes normalized bf16 tiles to DRAM scratch."""
        row0 = q * BQ + r * 128
        x = capin.tile([128, C], F32, tag="cap_in", name=f"cap_in_{q}_{r}")
        nc.sync.dma_start(x[:], cap_d[row0:row0 + 128, :])
        n128 = prep.tile([128, C], BF16, tag="cap_n128", name=f"cap_n128_{q}_{r}")
        n256 = prep.tile([128, C], BF16, tag="cap_n256", name=f"cap_n256_{q}_{r}")
        normalize_tile(x, n128, n256, f"cap{q}_{r}", mode=mode, pool=prep)
        if q == 0:
            for half, (srcT, dstq) in enumerate(((n128, capT128[0]), (n256, capT256[0]))):
                for jg in range(2):
                    pt = psum.tile([128, 4, 128], BF16, tag="u",
                                   name=f"pt_{q}_{r}_{half}_{jg}")
                    for k in range(4):
                        j = jg * 4 + k
                        nc.tensor.transpose(
                            pt[:, k, :], srcT[:, j * 128:(j + 1) * 128], ident[:]
                        )
                    dst = dstq[:, jg * 4:(jg + 1) * 4, r * 128:(r + 1) * 128]
                    if (half + jg) % 2 == 0:
                        nc.vector.tensor_copy(dst, pt[:])
                    else:
                        nc.scalar.copy(dst, pt[:])
        else:
            nc.sync.dma_start(scr_c128[row0:row0 + 128, :], n128[:])
            nc.sync.dma_start(scr_c256[row0:row0 + 128, :], n256[:])

    def cap_transpose_quarter(q):
        for j in range(8):
            nc.sync.dma_start_transpose(
                capT128[q][:, j, :],
                scr_c128[q * BQ:(q + 1) * BQ, j * 128:(j + 1) * 128])
            nc.sync.dma_start_transpose(
                capT256[q][:, j, :],
                scr_c256[q * BQ:(q + 1) * BQ, j * 128:(j + 1) * 128])

    # ---- steady-state cap prep (q1-3): 3-stage software pipeline.  Jobs
    # are emitted one per main-loop unit ("pumped") so every cross-engine
    # hop (GP square -> DVE reduce -> ACT rsqrt -> GP muls) is already
    # satisfied when the consumer engine reaches it -- no head-of-line
    # stalls in the ACT/DVE queues.
    import collections as _collections
    prep_jobs = _collections.deque()
    prep_state = {}

    def pump(n=1):
        for _ in range(n):
            if prep_jobs:
                prep_jobs.popleft()()

    def _stage1(q, r):
        def job():
            row0 = q * BQ + r * 128
            x = capin.tile([128, C], F32, tag="cap_in", name=f"cap_in_{q}_{r}")
            nc.sync.dma_start(x[:], cap_d[row0:row0 + 128, :])
            sq = norm_sums(x, f"cap{q}_{r}", sq_act=False)
            prep_state[(q, r)] = (x, sq)
        return job

    def _stage2(q, r):
        def job():
            x, sq = prep_state[(q, r)]
            s12 = norm_reduce(sq, f"cap{q}_{r}")
            prep_state[(q, r)] = (x, s12)
        return job

    def _stage3(q, r):
        def job():
            x, s12 = prep_state.pop((q, r))
            row0 = q * BQ + r * 128
            n128 = prep.tile([128, C], BF16, tag="cap_n128", name=f"cap_n128_{q}_{r}")
            n256 = prep.tile([128, C], BF16, tag="cap_n256", name=f"cap_n256_{q}_{r}")
            norm_apply(x, s12, n128, n256, f"cap{q}_{r}", mul_gp=True)
            nc.sync.dma_start(scr_c128[row0:row0 + 128, :], n128[:])
            nc.sync.dma_start(scr_c256[row0:row0 + 128, :], n256[:])
        return job

    capsR = {}

    def enqueue_prep(q):
        for r in range(4):
            prep_jobs.append(_stage1(q, r))
            prep_jobs.append(_stage2(q, r))
            prep_jobs.append(_stage3(q, r))
        prep_jobs.append(lambda: cap_transpose_quarter(q))
        prep_jobs.append(lambda: capsR.__setitem__(q, capsums(q)))

    def capsums(q):
        """cs128 = sum_t c128T[t]; cs256[h] = sum_tp c256T[2tp+h]."""
        c128q, c256q = capT128[q], capT256[q]
        cs128 = persist.tile([128, BQ], BF16, tag=f"cs128_{q % 2}", name=f"cs128_{q}")
        cs256 = persist.tile([128, 2, BQ], BF16, tag=f"cs256_{q % 2}", name=f"cs256_{q}")
        eng = nc.vector if q == 0 else nc.gpsimd
        f4 = csum.tile([128, 4, BQ], BF16, tag="csf4", name=f"csf4_{q}")
        eng.tensor_add(f4[:], c128q[:, 0:4, :], c128q[:, 4:8, :])
        f2 = csum.tile([128, 2, BQ], BF16, tag="csf2", name=f"csf2_{q}")
        eng.tensor_add(f2[:], f4[:, 0:2, :], f4[:, 2:4, :])
        eng.tensor_add(cs128[:], f2[:, 0, :], f2[:, 1, :])
        g4 = csum.tile([128, 2, 2, BQ], BF16, tag="csg4", name=f"csg4_{q}")
        eng.tensor_add(
            g4[:], c256q.rearrange("p (tp h) b -> p tp h b", h=2)[:, 0:2],
            c256q.rearrange("p (tp h) b -> p tp h b", h=2)[:, 2:4])
        eng.tensor_add(cs256[:], g4[:, 0], g4[:, 1])
        return cs128, cs256

    # ---------------- main loop ------------------------------------------------
    def main_quarter(q, mid_emit=None):
        # pending_tsum: one-element list in enclosing scope (deferred tsum)
        c128q, c256q = capT128[q], capT256[q]

        def do_tq(at, tq, stg):
            # p3 allocated first: its slot frees early (after the ACT
            # relu-drain to r3) and is reused by u2 in the same tq.
            p3 = psum.tile([128, 2, BQ], F32, tag="u", name=f"p3_{q}_{at}_{tq}")
            for ti in range(2):
                nc.tensor.matmul(p3[:, ti, :], w128T[at][:, 3, :],
                                 c128q[:, 2 * tq + ti, :], start=True, stop=True)
            u0 = psum.tile([128, 2, BQ], F32, tag="u", name=f"u0_{q}_{at}_{tq}")
            for ti in range(2):
                nc.tensor.matmul(u0[:, ti, :], w128T[at][:, 0, :],
                                 c128q[:, 2 * tq + ti, :], start=True, stop=True)
            r3 = drain.tile([128, 2, BQ], BF16, tag="r3", name=f"r3_{q}_{at}_{tq}")
            nc.scalar.activation(r3[:], p3[:], Act.Relu)
            u1 = psum.tile([128, 2, BQ], F32, tag="u", name=f"u1_{q}_{at}_{tq}")
            for ti in range(2):
                nc.tensor.matmul(u1[:, ti, :], w128T[at][:, 1, :],
                                 c128q[:, 2 * tq + ti, :], start=True, stop=True)
            nc.scalar.activation(u0[:], u0[:], Act.Relu)
            u2 = psum.tile([128, 2, BQ], F32, tag="u", name=f"u2_{q}_{at}_{tq}")
            for ti in range(2):
                nc.tensor.matmul(u2[:, ti, :], w128T[at][:, 2, :],
                                 c128q[:, 2 * tq + ti, :], start=True, stop=True)
            nc.scalar.activation(u1[:], u1[:], Act.Relu)
            for ti in range(2):
                nc.tensor.matmul(u0[:, ti, :], w128T[at][:, 4, :],
                                 c128q[:, 2 * tq + ti, :],
                                 start=False, stop=True, skip_group_check=True)
            nc.scalar.activation(u2[:], u2[:], Act.Relu)
            for p, u in ((1, u1), (2, u2)):
                for ti in range(2):
                    nc.tensor.matmul(u[:, ti, :], w128T[at][:, 4 + p, :],
                                     c128q[:, 2 * tq + ti, :],
                                     start=False, stop=True, skip_group_check=True)
            xm = drain.tile([128, 2, BQ], BF16, tag="xm", name=f"xm_{q}_{at}_{tq}")
            nc.vector.tensor_tensor(xm[:], u0[:], r3[:], op=Alu.max)
            ym = drain.tile([128, 2, BQ], BF16, tag="ym", name=f"ym_{q}_{at}_{tq}")
            nc.vector.tensor_tensor(ym[:], u1[:], xm[:], op=Alu.max)
            nc.vector.tensor_tensor(stg[:, tq], u2[:], ym[:], op=Alu.max)

        def do_tqp(at, tqp, stg):
            pc1 = psum.tile([128, 2, BQ], F32, tag="u", name=f"pc1_{q}_{at}_{tqp}")
            for tpi in range(2):
                for h in range(2):
                    nc.tensor.matmul(
                        pc1[:, tpi, :], w256T[at][:, 2 + h, :],
                        c256q[:, 2 * (2 * tqp + tpi) + h, :],
                        start=(h == 0), stop=(h == 1))
            pc0 = psum.tile([128, 2, BQ], F32, tag="u", name=f"pc0_{q}_{at}_{tqp}")
            for tpi in range(2):
                for h in range(2):
                    nc.tensor.matmul(
                        pc0[:, tpi, :], w256T[at][:, h, :],
                        c256q[:, 2 * (2 * tqp + tpi) + h, :],
                        start=(h == 0), stop=(h == 1))
            r1 = drain.tile([128, 2, BQ], BF16, tag="r1", name=f"r1_{q}_{at}_{tqp}")
            nc.scalar.activation(r1[:], pc1[:], Act.Relu)
            nc.scalar.activation(pc0[:], pc0[:], Act.Relu)
            for tpi in range(2):
                for h in range(2):
                    nc.tensor.matmul(
                        pc0[:, tpi, :], w256T[at][:, 4 + h, :],
                        c256q[:, 2 * (2 * tqp + tpi) + h, :],
                        start=False, stop=(h == 1), skip_group_check=True)
            nc.vector.tensor_tensor(stg[:, 4 + tqp], pc0[:], r1[:], op=Alu.max)

        def make_tsum(at, stg):
            # ---- t-sum, deferred: emitted after the NEXT block's first
            # tq's so these PE ops don't block seed matmuls while the
            # DVE max chain finishes writing stg ----
            asl = slice(at * 128, (at + 1) * 128)

            def emit():
                cs128, cs256 = capsR[q]
                acc = psacc.tile([128, BQ], F32, tag="acc", name=f"acc_{q}_{at}")
                nc.tensor.matmul(acc[:], w128T[at][:, 7, :], cs128[:],
                                 start=True, stop=False)
                for h in range(2):
                    nc.tensor.matmul(acc[:], w256T[at][:, 6 + h, :],
                                     cs256[:, h, :],
                                     start=False, stop=False,
                                     skip_group_check=True)
                for s in range(6):
                    for ti in range(2):
                        nc.tensor.matmul(acc[:], ident[:], stg[:, s, ti, :],
                                         start=False, stop=(s == 5 and ti == 1),
                                         skip_group_check=True)
                accs = drain.tile([128, BQ], F32, tag="accs",
                                  name=f"accs_{q}_{at}")
                if (2 * q + at) % 2 == 0:
                    nc.scalar.copy(accs[:], acc[:])
                else:
                    nc.vector.tensor_copy(accs[:], acc[:])
                nc.sync.dma_start(out_d[asl, q * BQ:(q + 1) * BQ], accs[:])
            return emit

        nonlocal_pending = pending_tsum[0]
        for at in range(2):
            if at == 1 and mid_emit is not None:
                mid_emit()
            stg = stage.tile([128, 6, 2, BQ], BF16, tag="stg", name=f"stg_{q}_{at}")
            # interleaved schedule smooths the Scalar/Vector mix
            do_tq(at, 0, stg)
            pump(2)
            do_tq(at, 1, stg)
            if nonlocal_pending is not None:
                nonlocal_pending()
            nonlocal_pending = None
            pump(2)
            do_tqp(at, 0, stg)
            pump(2)
            do_tq(at, 2, stg)
            pump(2)
            do_tq(at, 3, stg)
            pump(2)
            do_tqp(at, 1, stg)
            pump(2)
            nonlocal_pending = make_tsum(at, stg)
        pending_tsum[0] = nonlocal_pending

    # ---------------- schedule -------------------------------------------------
    # Prologue: interleave img prep (ACT/DVE) with cap q0 (alternating
    # fast/gp so neither GP nor ACT/DVE serializes the start).
    img_prep(0)
    cap_norm_tile(0, 0, mode="fast")
    cap_norm_tile(0, 1, mode="fast")
    img_prep(1)
    cap_norm_tile(0, 2, mode="fast")
    cap_norm_tile(0, 3, mode="fast")
    capsR[0] = capsums(0)

    enqueue_prep(1)
    pump(4)   # get q1 moving while the prologue finishes
    pending_tsum = [None]
    main_quarter(0, mid_emit=lambda: enqueue_prep(2))
    main_quarter(1, mid_emit=lambda: enqueue_prep(3))
    main_quarter(2)
    main_quarter(3)
    pending_tsum[0]()
    pump(len(prep_jobs))


_NC_CACHE = None


# ---------------------------------------------------------------------------
# Workaround: this container's walrus build rejects instructions with more
# than one sync-wait condition ("Too many sync wait commands").  Split the
# extra waits onto sequencer-only RegisterMove carrier instructions in a BIR
# post-pass, and monkeypatch the compile entry points to apply it.
import json as _json


def _split_multiwaits(bir_bytes: bytes) -> bytes:
    m = _json.loads(bir_bytes)
    uid = [0]

    def carrier(engine, wait, debug):
        uid[0] += 1
        return {
            "debug": debug,
            "engine": engine,
            "ins": [{"dtype": "int32", "kind": "imm_value", "value": 0}],
            "outs": [{"dtype": "int32", "kind": "register_access",
                      "regref": f"{engine}_zero"}],
            "name": f"I-wsplit-{uid[0]}",
            "opcode": "RegisterMove",
            "sync_info": {"on_update": [], "on_wait": [wait]},
        }

    for f in m["functions"]:
        for bb in f["blocks"]:
            out = []
            for inst in bb["instructions"]:
                si = inst.get("sync_info")
                waits = (si or {}).get("on_wait") or []
                eng = inst.get("engine")
                if len(waits) > 1 and eng and eng != "Unassigned":
                    for w in waits[:-1]:
                        out.append(carrier(eng, w, inst.get("debug", 0)))
                    si["on_wait"] = [waits[-1]]
                out.append(inst)
            bb["instructions"] = out
    return _json.dumps(m).encode()


def _install_birpatch():
    import concourse.bass_utils as bu
    import concourse.bass2jax as b2j

    if getattr(bu.compile_bir_kernel, "_wsplit_wrapped", False):
        return
    orig = bu.compile_bir_kernel

    def wrapped(bir_json: bytes, tmpdir: str, neff_name="file.neff"):
        return orig(_split_multiwaits(bir_json), tmpdir, neff_name=neff_name)

    wrapped._wsplit_wrapped = True
    bu.compile_bir_kernel = wrapped
    b2j.compile_bir_kernel = wrapped


def kernel(img_emb: np.ndarray, cap_emb: np.ndarray) -> np.ndarray:
    _install_birpatch()
    from concourse.bass_utils import run_bass_kernel_spmd

    global _NC_CACHE
    if _NC_CACHE is None:
        _NC_CACHE = _build_kernel()
    nc = _NC_CACHE

    import ml_dtypes

    img = np.ascontiguousarray(np.asarray(img_emb, dtype=np.float32))
    cap = np.ascontiguousarray(np.asarray(cap_emb, dtype=np.float32))
    ident = np.eye(128, dtype=ml_dtypes.bfloat16)
    in_maps = [
        {"img": img[k * A_PER:(k + 1) * A_PER], "cap": cap, "ident": ident}
        for k in range(N_CORES)
    ]
    res = run_bass_kernel_spmd(nc, in_maps, core_ids=list(range(N_CORES)))
    return np.concatenate([r["sims"] for r in res.results], axis=0)


if __name__ == "__main__":
    rng = np.random.default_rng(0)
    img = rng.normal(size=(A, C)).astype(np.float32)
    cap = rng.normal(size=(B, C)).astype(np.float32)
    out = kernel(img, cap)
    print("out", out.shape, out.dtype, float(out.min()), float(out.max()))


# revision 16
# speedup vs baseline: 1.0530x; 1.0530x over previous
"""Trainium2 Bass kernel for nn_EncoderSimilarity (block-cosine similarity).

sims[a,b] = sum over block-granularities {128, 256} of
            sum_t max_v ( l2norm(img_block_v) . l2norm(cap_block_t) )

Sharding: img rows (axis a) split 8 ways across cores, cap replicated;
each core computes its [256, 2048] slice of sims.

v3 device algorithm (per core).  Same math as v2 (max-of-8 restructured
into relu'd pair differences + PSUM-accumulated deltas; t-sums factored
through the PE via capsums and identity matmuls), re-engineered for
engine overlap:

  * PSUM is carved into 2-bank unit tiles ([128,2,512] f32) from a
    bufs=3 rotation (6 banks) + 1 acc bank + 1 transpose bank.  p3 is
    allocated FIRST in each tq so u2 reuses its slot right after the
    early ACT relu-drain; the PE streams tq(k+1) seeds while ACT/DVE
    drain tq(k).  v2 used all 8 banks per tq -> no cross-tq overlap.
  * Prep engine assignment keeps the DVE/ACT main path free of
    head-of-line blocking: q1-3 normalization (squares, reduces,
    scale-muls) runs entirely on GPSIMD, q0+img prep splits across
    ACT/DVE/GP in the prologue, capsums q0 on GP / q1-3 on DVE emitted
    one quarter ahead of use.
"""
import sys

if "/opt/trn_rl_repo" not in sys.path:
    sys.path.insert(0, "/opt/trn_rl_repo")

from contextlib import ExitStack

import numpy as np

N_CORES = 8
A, B, C = 2048, 2048, 1024
A_PER = A // N_CORES          # 256 img rows per core
NQ = 4                        # b processed in quarters of 512
BQ = B // NQ                  # 512


def _build_kernel():
    import concourse.bass as bass
    import concourse.tile as tile
    from concourse import mybir

    F32 = mybir.dt.float32
    BF16 = mybir.dt.bfloat16
    Alu = mybir.AluOpType
    Act = mybir.ActivationFunctionType
    Ax = mybir.AxisListType

    nc = bass.Bass(
        trn_type="TRN2",
        target_bir_lowering=False,
        debug=False,
        num_devices=N_CORES,
    )
    img_d = nc.dram_tensor("img", [A_PER, C], F32, kind="ExternalInput").ap()
    cap_d = nc.dram_tensor("cap", [B, C], F32, kind="ExternalInput").ap()
    ident_d = nc.dram_tensor("ident", [128, 128], BF16, kind="ExternalInput").ap()
    out_d = nc.dram_tensor("sims", [A_PER, B], F32, kind="ExternalOutput").ap()

    with tile.TileContext(nc) as tc, ExitStack() as ctx:
        _body(ctx, tc, out_d, img_d, cap_d, ident_d, F32, BF16, Alu, Act, Ax)
    return nc


def _body(ctx, tc, out_d, img_d, cap_d, ident_d, F32, BF16, Alu, Act, Ax):
    nc = tc.nc

    dram = ctx.enter_context(tc.tile_pool(name="dram", bufs=1, space="DRAM"))
    persist = ctx.enter_context(tc.tile_pool(name="persist", bufs=1))
    norm = ctx.enter_context(tc.tile_pool(name="norm", bufs=2))
    small = ctx.enter_context(tc.tile_pool(name="small", bufs=3))
    stage = ctx.enter_context(tc.tile_pool(name="stage", bufs=2))
    drain = ctx.enter_context(tc.tile_pool(name="drain", bufs=2))
    csum = ctx.enter_context(tc.tile_pool(name="csum", bufs=1))
    prep = ctx.enter_context(tc.tile_pool(name="prep", bufs=3))
    capin = ctx.enter_context(tc.tile_pool(name="capin", bufs=6))
    # PSUM: "u" 3 x 2 banks + acc 1 bank + transpose 1 bank = 8 banks
    psum = ctx.enter_context(tc.tile_pool(name="psum", bufs=3, space="PSUM"))
    psacc = ctx.enter_context(tc.tile_pool(name="psacc", bufs=2, space="PSUM"))

    ident = persist.tile([128, 128], BF16, tag="ident")
    nc.sync.dma_start(ident[:], ident_d[:])

    # ---------------- normalization helper (natural [n, c] layout) -------------
    def norm_sums(x_f32, nm, sq_act):
        """square + per-block sums -> sq tile.  Separate tile tags per path:
        sharing a ring between the prologue (ACT/DVE) and steady-state (GP)
        paths creates write-after-read hazards that stall GP behind the
        whole prologue DVE chain."""
        tag = "sq" if sq_act else "sqg"
        sq = norm.tile([128, C], F32, tag=tag, name=f"sq_{nm}", bufs=2)
        if sq_act:
            nc.scalar.activation(sq[:], x_f32[:], Act.Square)
        else:
            nc.gpsimd.tensor_mul(sq[:], x_f32[:], x_f32[:])
        return sq

    def norm_reduce(sq, nm):
        s12 = small.tile([128, 12], F32, tag="s12", name=f"s12_{nm}")
        nc.vector.reduce_sum(
            s12[:, 0:8], sq.rearrange("p (j c) -> p j c", c=128), axis=Ax.X
        )
        nc.vector.tensor_tensor(
            s12[:, 8:12],
            s12.rearrange("p (k two) -> p k two", two=2)[:, 0:4, 0],
            s12.rearrange("p (k two) -> p k two", two=2)[:, 0:4, 1],
            op=Alu.add,
        )
        return s12

    def norm_apply(x_f32, s12, n128_out, n256_out, nm, mul_gp):
        rt = small.tile([128, 12], F32, tag="rt", name=f"rt_{nm}")
        nc.scalar.activation(rt[:], s12[:], Act.Sqrt)
        inv = small.tile([128, 12], F32, tag="inv", name=f"inv_{nm}")
        nc.vector.reciprocal(inv[:], rt[:])
        eng = nc.gpsimd if mul_gp else nc.vector
        eng.tensor_mul(
            n128_out.rearrange("p (j c) -> p j c", c=128),
            x_f32.rearrange("p (j c) -> p j c", c=128),
            inv[:, 0:8].unsqueeze(2).to_broadcast((128, 8, 128)),
        )
        eng.tensor_mul(
            n256_out.rearrange("p (k c) -> p k c", c=256),
            x_f32.rearrange("p (k c) -> p k c", c=256),
            inv[:, 8:12].unsqueeze(2).to_broadcast((128, 4, 256)),
        )

    def normalize_tile(x_f32, n128_out, n256_out, nm, mode, pool=None):
        """Prologue path: all three phases back-to-back."""
        sq = norm_sums(x_f32, nm, sq_act=True)
        s12 = norm_reduce(sq, nm)
        norm_apply(x_f32, s12, n128_out, n256_out, nm, mul_gp=(mode == "gp"))

    # ---------------- img prep -> transposed bf16 weight tiles -----------------
    # w128T slots: 0..3 = D_p = n128[2p]-n128[2p+1]; 4..6 = wd_j = n128[2j+1]-
    # n128[7]; 7 = base n128[7].
    # w256T slots: 0..3 = D'_i half h (2i+h); 4..5 = wd' h; 6..7 = base' h.
    w128T = [persist.tile([128, 8, 128], BF16, tag=f"w128T_{at}", name=f"w128T_{at}") for at in range(2)]
    w256T = [persist.tile([128, 8, 128], BF16, tag=f"w256T_{at}", name=f"w256T_{at}") for at in range(2)]

    def img_prep(at):
        x = norm.tile([128, C], F32, tag="img_in", name=f"img_in_{at}")
        nc.sync.dma_start(x[:], img_d[at * 128:(at + 1) * 128, :])
        n128 = norm.tile([128, C], BF16, tag="img_n128", name=f"img_n128_{at}")
        n256 = norm.tile([128, C], BF16, tag="img_n256", name=f"img_n256_{at}")
        normalize_tile(x, n128, n256, f"img{at}", mode="fast")

        d128 = norm.tile([128, 8, 128], BF16, tag="d128", name=f"d128_{at}")
        v128 = n128.rearrange("p (v c) -> p v c", c=128)
        nc.vector.tensor_tensor(d128[:, 0:4, :], v128[:, 0::2, :], v128[:, 1::2, :],
                                op=Alu.subtract)
        nc.vector.tensor_tensor(
            d128[:, 4:7, :], v128[:, 1:7:2, :],
            v128[:, 7:8, :].to_broadcast((128, 3, 128)), op=Alu.subtract)
        nc.vector.tensor_copy(d128[:, 7, :], v128[:, 7, :])

        d256 = norm.tile([128, 8, 128], BF16, tag="d256", name=f"d256_{at}")
        v256 = n256.rearrange("p (v c) -> p v c", c=256)
        nc.vector.tensor_tensor(
            d256.rearrange("p (i h) c -> p i (h c)", h=2)[:, 0:2, :],
            v256[:, 0::2, :], v256[:, 1::2, :], op=Alu.subtract)
        nc.vector.tensor_tensor(d256[:, 4:6, :].rearrange("p h c -> p (h c)"),
                                v256[:, 1, :], v256[:, 3, :], op=Alu.subtract)
        nc.vector.tensor_copy(d256[:, 6:8, :].rearrange("p h c -> p (h c)"),
                              v256[:, 3, :])

        for gi, (src, dstT) in enumerate(((d128, w128T[at]), (d256, w256T[at]))):
            for jg in range(2):
                pt = psum.tile([128, 4, 128], BF16, tag="u",
                               name=f"ptw_{at}_{gi}_{jg}")
                for k in range(4):
                    nc.tensor.transpose(pt[:, k, :], src[:, jg * 4 + k, :], ident[:])
                if (gi + jg) % 2 == 0:
                    nc.vector.tensor_copy(dstT[:, jg * 4:(jg + 1) * 4, :], pt[:])
                else:
                    nc.scalar.copy(dstT[:, jg * 4:(jg + 1) * 4, :], pt[:])

    # ---------------- cap prep ------------------------------------------------
    scr_c128 = dram.tile([B, C], BF16, tag="scr_c128")
    scr_c256 = dram.tile([B, C], BF16, tag="scr_c256")

    capT128 = {}
    capT256 = {}
    for q in range(NQ):
        capT128[q] = persist.tile([128, 8, BQ], BF16, tag=f"capT128_{q % 2}",
                                  name=f"capT128_{q}")
        capT256[q] = persist.tile([128, 8, BQ], BF16, tag=f"capT256_{q % 2}",
                                  name=f"capT256_{q}")

    def cap_norm_tile(q, r, mode):
        """Normalize cap row-tile (q, r); q0 PE-transposes from SBUF,
        q>0 writes normalized bf16 tiles to DRAM scratch."""
        row0 = q * BQ + r * 128
        x = capin.tile([128, C], F32, tag="cap_in", name=f"cap_in_{q}_{r}")
        nc.sync.dma_start(x[:], cap_d[row0:row0 + 128, :])
        n128 = prep.tile([128, C], BF16, tag="cap_n128", name=f"cap_n128_{q}_{r}")
        n256 = prep.tile([128, C], BF16, tag="cap_n256", name=f"cap_n256_{q}_{r}")
        normalize_tile(x, n128, n256, f"cap{q}_{r}", mode=mode, pool=prep)
        if q == 0:
            for half, (srcT, dstq) in enumerate(((n128, capT128[0]), (n256, capT256[0]))):
                for jg in range(2):
                    pt = psum.tile([128, 4, 128], BF16, tag="u",
                                   name=f"pt_{q}_{r}_{half}_{jg}")
                    for k in range(4):
                        j = jg * 4 + k
                        nc.tensor.transpose(
                            pt[:, k, :], srcT[:, j * 128:(j + 1) * 128], ident[:]
                        )
                    dst = dstq[:, jg * 4:(jg + 1) * 4, r * 128:(r + 1) * 128]
                    if (half + jg) % 2 == 0:
                        nc.vector.tensor_copy(dst, pt[:])
                    else:
                        nc.scalar.copy(dst, pt[:])
        else:
            nc.sync.dma_start(scr_c128[row0:row0 + 128, :], n128[:])
            nc.sync.dma_start(scr_c256[row0:row0 + 128, :], n256[:])

    def cap_transpose_quarter(q):
        for j in range(8):
            nc.sync.dma_start_transpose(
                capT128[q][:, j, :],
                scr_c128[q * BQ:(q + 1) * BQ, j * 128:(j + 1) * 128])
            nc.sync.dma_start_transpose(
                capT256[q][:, j, :],
                scr_c256[q * BQ:(q + 1) * BQ, j * 128:(j + 1) * 128])

    # ---- steady-state cap prep (q1-3): 3-stage software pipeline.  Jobs
    # are emitted one per main-loop unit ("pumped") so every cross-engine
    # hop (GP square -> DVE reduce -> ACT rsqrt -> GP muls) is already
    # satisfied when the consumer engine reaches it -- no head-of-line
    # stalls in the ACT/DVE queues.
    import collections as _collections
    prep_jobs = _collections.deque()
    prep_state = {}

    def pump(n=1):
        for _ in range(n):
            if prep_jobs:
                prep_jobs.popleft()()

    def _stage1(q, r):
        def job():
            row0 = q * BQ + r * 128
            x = capin.tile([128, C], F32, tag="cap_in", name=f"cap_in_{q}_{r}")
            nc.sync.dma_start(x[:], cap_d[row0:row0 + 128, :])
            sq = norm_sums(x, f"cap{q}_{r}", sq_act=True)
            prep_state[(q, r)] = (x, sq)
        return job

    def _stage2(q, r):
        def job():
            x, sq = prep_state[(q, r)]
            s12 = norm_reduce(sq, f"cap{q}_{r}")
            prep_state[(q, r)] = (x, s12)
        return job

    def _stage3(q, r):
        def job():
            x, s12 = prep_state.pop((q, r))
            row0 = q * BQ + r * 128
            n128 = prep.tile([128, C], BF16, tag="cap_n128", name=f"cap_n128_{q}_{r}")
            n256 = prep.tile([128, C], BF16, tag="cap_n256", name=f"cap_n256_{q}_{r}")
            norm_apply(x, s12, n128, n256, f"cap{q}_{r}", mul_gp=True)
            nc.sync.dma_start(scr_c128[row0:row0 + 128, :], n128[:])
            nc.sync.dma_start(scr_c256[row0:row0 + 128, :], n256[:])
        return job

    capsR = {}

    def enqueue_prep(q):
        for r in range(4):
            prep_jobs.append(_stage1(q, r))
            prep_jobs.append(_stage2(q, r))
            prep_jobs.append(_stage3(q, r))
        prep_jobs.append(lambda: cap_transpose_quarter(q))
        prep_jobs.append(lambda: capsR.__setitem__(q, capsums(q)))

    def capsums(q):
        """cs128 = sum_t c128T[t]; cs256[h] = sum_tp c256T[2tp+h]."""
        c128q, c256q = capT128[q], capT256[q]
        cs128 = persist.tile([128, BQ], BF16, tag=f"cs128_{q % 2}", name=f"cs128_{q}")
        cs256 = persist.tile([128, 2, BQ], BF16, tag=f"cs256_{q % 2}", name=f"cs256_{q}")
        eng = nc.vector if q == 0 else nc.gpsimd
        f4 = csum.tile([128, 4, BQ], BF16, tag="csf4", name=f"csf4_{q}")
        eng.tensor_add(f4[:], c128q[:, 0:4, :], c128q[:, 4:8, :])
        f2 = csum.tile([128, 2, BQ], BF16, tag="csf2", name=f"csf2_{q}")
        eng.tensor_add(f2[:], f4[:, 0:2, :], f4[:, 2:4, :])
        eng.tensor_add(cs128[:], f2[:, 0, :], f2[:, 1, :])
        g4 = csum.tile([128, 2, 2, BQ], BF16, tag="csg4", name=f"csg4_{q}")
        eng.tensor_add(
            g4[:], c256q.rearrange("p (tp h) b -> p tp h b", h=2)[:, 0:2],
            c256q.rearrange("p (tp h) b -> p tp h b", h=2)[:, 2:4])
        eng.tensor_add(cs256[:], g4[:, 0], g4[:, 1])
        return cs128, cs256

    # ---------------- main loop ------------------------------------------------
    def main_quarter(q, mid_emit=None):
        # pending_tsum: one-element list in enclosing scope (deferred tsum)
        c128q, c256q = capT128[q], capT256[q]

        def do_tq(at, tq, stg):
            # p3 allocated first: its slot frees early (after the ACT
            # relu-drain to r3) and is reused by u2 in the same tq.
            p3 = psum.tile([128, 2, BQ], F32, tag="u", name=f"p3_{q}_{at}_{tq}")
            for ti in range(2):
                nc.tensor.matmul(p3[:, ti, :], w128T[at][:, 3, :],
                                 c128q[:, 2 * tq + ti, :], start=True, stop=True)
            u0 = psum.tile([128, 2, BQ], F32, tag="u", name=f"u0_{q}_{at}_{tq}")
            for ti in range(2):
                nc.tensor.matmul(u0[:, ti, :], w128T[at][:, 0, :],
                                 c128q[:, 2 * tq + ti, :], start=True, stop=True)
            r3 = drain.tile([128, 2, BQ], BF16, tag="r3", name=f"r3_{q}_{at}_{tq}")
            nc.scalar.activation(r3[:], p3[:], Act.Relu)
            u1 = psum.tile([128, 2, BQ], F32, tag="u", name=f"u1_{q}_{at}_{tq}")
            for ti in range(2):
                nc.tensor.matmul(u1[:, ti, :], w128T[at][:, 1, :],
                                 c128q[:, 2 * tq + ti, :], start=True, stop=True)
            nc.scalar.activation(u0[:], u0[:], Act.Relu)
            u2 = psum.tile([128, 2, BQ], F32, tag="u", name=f"u2_{q}_{at}_{tq}")
            for ti in range(2):
                nc.tensor.matmul(u2[:, ti, :], w128T[at][:, 2, :],
                                 c128q[:, 2 * tq + ti, :], start=True, stop=True)
            nc.scalar.activation(u1[:], u1[:], Act.Relu)
            for ti in range(2):
                nc.tensor.matmul(u0[:, ti, :], w128T[at][:, 4, :],
                                 c128q[:, 2 * tq + ti, :],
                                 start=False, stop=True, skip_group_check=True)
            nc.scalar.activation(u2[:], u2[:], Act.Relu)
            for p, u in ((1, u1), (2, u2)):
                for ti in range(2):
                    nc.tensor.matmul(u[:, ti, :], w128T[at][:, 4 + p, :],
                                     c128q[:, 2 * tq + ti, :],
                                     start=False, stop=True, skip_group_check=True)
            xm = drain.tile([128, 2, BQ], BF16, tag="xm", name=f"xm_{q}_{at}_{tq}")
            nc.vector.tensor_tensor(xm[:], u0[:], r3[:], op=Alu.max)
            ym = drain.tile([128, 2, BQ], BF16, tag="ym", name=f"ym_{q}_{at}_{tq}")
            nc.vector.tensor_tensor(ym[:], u1[:], xm[:], op=Alu.max)
            nc.vector.tensor_tensor(stg[:, tq], u2[:], ym[:], op=Alu.max)

        def do_tqp(at, tqp, stg):
            pc1 = psum.tile([128, 2, BQ], F32, tag="u", name=f"pc1_{q}_{at}_{tqp}")
            for tpi in range(2):
                for h in range(2):
                    nc.tensor.matmul(
                        pc1[:, tpi, :], w256T[at][:, 2 + h, :],
                        c256q[:, 2 * (2 * tqp + tpi) + h, :],
                        start=(h == 0), stop=(h == 1))
            pc0 = psum.tile([128, 2, BQ], F32, tag="u", name=f"pc0_{q}_{at}_{tqp}")
            for tpi in range(2):
                for h in range(2):
                    nc.tensor.matmul(
                        pc0[:, tpi, :], w256T[at][:, h, :],
                        c256q[:, 2 * (2 * tqp + tpi) + h, :],
                        start=(h == 0), stop=(h == 1))
            r1 = drain.tile([128, 2, BQ], BF16, tag="r1", name=f"r1_{q}_{at}_{tqp}")
            nc.scalar.activation(r1[:], pc1[:], Act.Relu)
            nc.scalar.activation(pc0[:], pc0[:], Act.Relu)
            for tpi in range(2):
                for h in range(2):
                    nc.tensor.matmul(
                        pc0[:, tpi, :], w256T[at][:, 4 + h, :],
                        c256q[:, 2 * (2 * tqp + tpi) + h, :],
                        start=False, stop=(h == 1), skip_group_check=True)
            nc.vector.tensor_tensor(stg[:, 4 + tqp], pc0[:], r1[:], op=Alu.max)

        def make_tsum(at, stg):
            # ---- t-sum, deferred: emitted after the NEXT block's first
            # tq's so these PE ops don't block seed matmuls while the
            # DVE max chain finishes writing stg ----
            asl = slice(at * 128, (at + 1) * 128)

            def emit():
                cs128, cs256 = capsR[q]
                acc = psacc.tile([128, BQ], F32, tag="acc", name=f"acc_{q}_{at}")
                nc.tensor.matmul(acc[:], w128T[at][:, 7, :], cs128[:],
                                 start=True, stop=False)
                for h in range(2):
                    nc.tensor.matmul(acc[:], w256T[at][:, 6 + h, :],
                                     cs256[:, h, :],
                                     start=False, stop=False,
                                     skip_group_check=True)
                for s in range(6):
                    for ti in range(2):
                        nc.tensor.matmul(acc[:], ident[:], stg[:, s, ti, :],
                                         start=False, stop=(s == 5 and ti == 1),
                                         skip_group_check=True)
                accs = drain.tile([128, BQ], F32, tag="accs",
                                  name=f"accs_{q}_{at}")
                if (2 * q + at) % 2 == 0:
                    nc.scalar.copy(accs[:], acc[:])
                else:
                    nc.vector.tensor_copy(accs[:], acc[:])
                nc.sync.dma_start(out_d[asl, q * BQ:(q + 1) * BQ], accs[:])
            return emit

        nonlocal_pending = pending_tsum[0]
        for at in range(2):
            if at == 1 and mid_emit is not None:
                mid_emit()
            stg = stage.tile([128, 6, 2, BQ], BF16, tag="stg", name=f"stg_{q}_{at}")
            # interleaved schedule smooths the Scalar/Vector mix
            do_tq(at, 0, stg)
            pump(2)
            do_tq(at, 1, stg)
            if nonlocal_pending is not None:
                nonlocal_pending()
            nonlocal_pending = None
            pump(2)
            do_tqp(at, 0, stg)
            pump(2)
            do_tq(at, 2, stg)
            pump(2)
            do_tq(at, 3, stg)
            pump(2)
            do_tqp(at, 1, stg)
            pump(2)
            nonlocal_pending = make_tsum(at, stg)
        pending_tsum[0] = nonlocal_pending

    # ---------------- schedule -------------------------------------------------
    # Prologue: interleave img prep (ACT/DVE) with cap q0 (alternating
    # fast/gp so neither GP nor ACT/DVE serializes the start).
    enqueue_prep(1)
    img_prep(0)
    pump(2)
    cap_norm_tile(0, 0, mode="fast")
    cap_norm_tile(0, 1, mode="fast")
    pump(2)
    img_prep(1)
    cap_norm_tile(0, 2, mode="fast")
    pump(2)
    cap_norm_tile(0, 3, mode="fast")
    capsR[0] = capsums(0)

    pending_tsum = [None]
    main_quarter(0, mid_emit=lambda: enqueue_prep(2))
    main_quarter(1, mid_emit=lambda: enqueue_prep(3))
    main_quarter(2)
    main_quarter(3)
    pending_tsum[0]()
    pump(len(prep_jobs))


_NC_CACHE = None


# ---------------------------------------------------------------------------
# Workaround: this container's walrus build rejects instructions with more
# than one sync-wait condition ("Too many sync wait commands").  Split the
# extra waits onto sequencer-only RegisterMove carrier instructions in a BIR
# post-pass, and monkeypatch the compile entry points to apply it.
import json as _json


def _split_multiwaits(bir_bytes: bytes) -> bytes:
    m = _json.loads(bir_bytes)
    uid = [0]

    def carrier(engine, wait, debug):
        uid[0] += 1
        return {
            "debug": debug,
            "engine": engine,
            "ins": [{"dtype": "int32", "kind": "imm_value", "value": 0}],
            "outs": [{"dtype": "int32", "kind": "register_access",
                      "regref": f"{engine}_zero"}],
            "name": f"I-wsplit-{uid[0]}",
            "opcode": "RegisterMove",
            "sync_info": {"on_update": [], "on_wait": [wait]},
        }

    for f in m["functions"]:
        for bb in f["blocks"]:
            out = []
            for inst in bb["instructions"]:
                si = inst.get("sync_info")
                waits = (si or {}).get("on_wait") or []
                eng = inst.get("engine")
                if len(waits) > 1 and eng and eng != "Unassigned":
                    for w in waits[:-1]:
                        out.append(carrier(eng, w, inst.get("debug", 0)))
                    si["on_wait"] = [waits[-1]]
                out.append(inst)
            bb["instructions"] = out
    return _json.dumps(m).encode()


def _install_birpatch():
    import concourse.bass_utils as bu
    import concourse.bass2jax as b2j

    if getattr(bu.compile_bir_kernel, "_wsplit_wrapped", False):
        return
    orig = bu.compile_bir_kernel

    def wrapped(bir_json: bytes, tmpdir: str, neff_name="file.neff"):
        return orig(_split_multiwaits(bir_json), tmpdir, neff_name=neff_name)

    wrapped._wsplit_wrapped = True
    bu.compile_bir_kernel = wrapped
    b2j.compile_bir_kernel = wrapped


def kernel(img_emb: np.ndarray, cap_emb: np.ndarray) -> np.ndarray:
    _install_birpatch()
    from concourse.bass_utils import run_bass_kernel_spmd

    global _NC_CACHE
    if _NC_CACHE is None:
        _NC_CACHE = _build_kernel()
    nc = _NC_CACHE

    import ml_dtypes

    img = np.ascontiguousarray(np.asarray(img_emb, dtype=np.float32))
    cap = np.ascontiguousarray(np.asarray(cap_emb, dtype=np.float32))
    ident = np.eye(128, dtype=ml_dtypes.bfloat16)
    in_maps = [
        {"img": img[k * A_PER:(k + 1) * A_PER], "cap": cap, "ident": ident}
        for k in range(N_CORES)
    ]
    res = run_bass_kernel_spmd(nc, in_maps, core_ids=list(range(N_CORES)))
    return np.concatenate([r["sims"] for r in res.results], axis=0)


if __name__ == "__main__":
    rng = np.random.default_rng(0)
    img = rng.normal(size=(A, C)).astype(np.float32)
    cap = rng.normal(size=(B, C)).astype(np.float32)
    out = kernel(img, cap)
    print("out", out.shape, out.dtype, float(out.min()), float(out.max()))
